# revision 25
# baseline (speedup 1.0000x reference)
"""Trainium2 Bass kernel for MemoryEfficientISNE GNN message passing (v2).

Full inputs in, full output out. 8-way data-parallel over nodes with a
balanced node->(core, bucket, slot) permutation computed on the host.

v2 design ("z-space" aggregation):
  - Phase A: 3-layer MLP in dim-major layout (fp32r matmuls, matmul-based
    LayerNorm stats), then z = h2 @ W3 (no LN), s = h2@wr + b_att, t = h2@wc.
    Emits per-node 768B rows [z bf16 x256 | t bf16 | pad] to ag_in, and
    2*z node-major f32 to zN_dram.
  - Chunked AllGather (4 chunks on bucket-group boundaries) of the rows,
    overlapping the phase A tail.
  - Phase B: per destination bucket, dma_gather 2x640 edge rows (lo/hi
    halves). Attention: one PE matmul broadcasts s over partitions; ten
    sigmoid activations add the per-edge t as per-partition bias; a host
    precomputed one-hot (dest routing) is DMA'd and multiplied in; the
    aggregation is 10 accumulating [slot,dst]^T x [slot,z] matmuls into a
    [dst, 256] PSUM tile. Final LayerNorm runs node-major on vector/scalar
    (LN(2*z + agg) == LN(z + 0.5*agg) by LN scale invariance).

Self-contained: hardcodes the problem shapes from the task spec.
"""
from dataclasses import dataclass, replace

import numpy as np

import concourse.bacc as bacc
import concourse.bass as bass
import concourse.tile as tile
from concourse import mybir
from concourse.bass_utils import run_bass_kernel_spmd
from concourse.masks import make_identity

f32 = mybir.dt.float32
f32r = mybir.dt.float32r
bf16 = mybir.dt.bfloat16
i16 = mybir.dt.int16
AF = mybir.ActivationFunctionType
ALU = mybir.AluOpType

LN_EPS = 1e-5
P = 128


@dataclass(frozen=True)
class Cfg:
    n_cores: int = 8
    d_in: int = 256          # D
    d_hid: int = 512         # H
    shard: int = 6272        # padded nodes per core (= buckets * 128)
    cpb: int = 10            # gather chunks per bucket (5 lo + 5 hi)
    row: int = 384           # ag row in bf16 elems (768B, %256==0)
    g_bufs: int = 4          # gather destination buffers in SBUF
    trace: bool = False
    b_att: float = 0.0

    @property
    def buckets(self):
        return self.shard // P

    @property
    def totn(self):
        return self.n_cores * self.shard

    @property
    def half_slots(self):
        return (self.cpb // 2) * P

    n_ag_chunks: int = 1

    # AllGather chunking: global row of (core c, slot s in chunk k) =
    # chunk_row_base[k] + c*chunk_rows[k] + (s - 128*bucket_base[k]).
    @property
    def bucket_base(self):
        if self.n_ag_chunks == 1:
            return [0, self.buckets]
        if self.n_ag_chunks == 2:
            return [0, 24, self.buckets]
        return [0, 12, 24, 36, self.buckets]

    @property
    def chunk_rows(self):
        bb = self.bucket_base
        return [(bb[k + 1] - bb[k]) * P for k in range(len(bb) - 1)]

    @property
    def chunk_row_base(self):
        out = [0]
        for r in self.chunk_rows:
            out.append(out[-1] + r * self.n_cores)
        return out

    @property
    def half_rows(self):
        # both halves must stay < 32768 rows for int16 gather indices
        if self.n_ag_chunks == 1:
            return self.totn // 2
        return self.chunk_row_base[len(self.bucket_base) // 2]


CFG = Cfg()


def build(cfg: Cfg, cnts=None):
    nc = bacc.Bacc("TRN2", target_bir_lowering=False, debug=False,
                   num_devices=cfg.n_cores)
    D, H, S = cfg.d_in, cfg.d_hid, cfg.shard
    B, CPB, R = cfg.buckets, cfg.cpb, cfg.row
    KD, KH = D // P, H // P
    HS16 = cfg.half_slots // 16

    # ---- I/O ----
    featT = nc.dram_tensor("featT", [D, S], f32, kind="ExternalInput").ap()
    embT = nc.dram_tensor("embT", [D, S], f32, kind="ExternalInput").ap()
    w_in = [nc.dram_tensor(f"w{i}", shp, f32, kind="ExternalInput").ap()
            for i, shp in enumerate([[D, H], [H, H], [H, H], [H, D]])]
    watt = nc.dram_tensor("watt", [H, 2], f32, kind="ExternalInput").ap()
    rs_in = [nc.dram_tensor(f"rs{i}", [[D, 1], [H, 1], [H, 1]][i], f32,
                            kind="ExternalInput").ap() for i in range(3)]
    dgidx = nc.dram_tensor("dgidx", [P, B * 2 * HS16], i16,
                           kind="ExternalInput").ap()
    ohT = nc.dram_tensor("ohT", [P, B * CPB * P], bf16,
                         kind="ExternalInput").ap()
    mbias = nc.dram_tensor("mbias", [1, 8], f32, kind="ExternalInput").ap()

    outN = nc.dram_tensor("outN", [S, D], f32, kind="ExternalOutput").ap()

    # ---- internal DRAM ----
    ag_in = nc.dram_tensor("ag_in", [S, R], bf16, kind="Internal").ap()
    ag_out = nc.dram_tensor(
        "ag_out", [cfg.totn, R], bf16, kind="Internal",
        addr_space="Shared" if cfg.n_cores > 4 else "Local").ap()
    zN_dram = nc.dram_tensor("zN_dram", [S, D], f32, kind="Internal").ap()

    with tile.TileContext(nc) as tc:
        with (
            tc.tile_pool(name="consts", bufs=1) as consts,
            tc.tile_pool(name="wstage", bufs=2) as wstage,
        ):
            def load_r(src_ap, shape, name):
                t_f = wstage.tile([P, 512], f32, tag="wstage")
                nc.sync.dma_start(t_f[:shape[0], :shape[1]], src_ap)
                t_r = consts.tile(shape, f32r, tag=name)
                nc.vector.tensor_copy(t_r[:], t_f[:shape[0], :shape[1]])
                return t_r

            w_r = []
            for i, w in enumerate(w_in):
                kin = w.shape[0] // P
                w_r.append([load_r(w[kt * P:(kt + 1) * P, :], [P, w.shape[1]],
                                   f"w{i}_{kt}") for kt in range(kin)])
            wattr = [load_r(watt[kt * P:(kt + 1) * P, :], [P, 2], f"watt_{kt}")
                     for kt in range(KH)]
            rs_r = []
            for i, rs in enumerate(rs_in):
                kin = rs.shape[0] // P
                rs_r.append([load_r(rs[kt * P:(kt + 1) * P, :], [P, 1],
                                    f"rs{i}_{kt}") for kt in range(kin)])

            ones_f = consts.tile([P, P], f32, tag="ones_f")
            nc.vector.memset(ones_f[:], 1.0)
            ones_row = consts.tile([1, P], f32r, tag="ones_row")
            nc.vector.tensor_copy(ones_row[:], ones_f[:1, :])
            ones_row_bf = consts.tile([1, P], bf16, tag="ones_row_bf")
            nc.vector.tensor_copy(ones_row_bf[:], ones_f[:1, :])
            ones_1 = consts.tile([1, 2], f32r, tag="ones_1")
            nc.vector.tensor_copy(ones_1[:], ones_f[:1, :2])
            ones_col = consts.tile([P, 1], f32r, tag="ones_col")
            nc.vector.tensor_copy(ones_col[:], ones_f[:, :1])

            ident_f = consts.tile([P, P], f32, tag="ident_f")
            make_identity(nc, ident_f[:])
            identr = consts.tile([P, P], f32r, tag="identr")
            nc.vector.tensor_copy(identr[:], ident_f[:])

            idx_sb = consts.tile([P, B * 2 * HS16], i16, tag="idx_sb")
            nc.sync.dma_start(idx_sb[:], dgidx)

            mb_sb = consts.tile([1, 8], f32, tag="mb_sb")
            nc.sync.dma_start(mb_sb[:], mbias)
            eps_t = consts.tile([1, 1], f32, tag="eps_t")
            nc.vector.memset(eps_t[:], LN_EPS)
            eps_col = consts.tile([P, 1], f32, tag="eps_col")
            nc.vector.memset(eps_col[:], LN_EPS)
            batt_sb = consts.tile([1, 1], f32, tag="batt_sb")
            nc.vector.memset(batt_sb[:], float(cfg.b_att))

            s_row = consts.tile([1, S], bf16, tag="s_row")

            # --------------------------------------------------------------
            def layer(sbp, psp, psp_y, x_tiles, li, n_out, T, relu):
                """One W@x + LayerNorm (+relu) in dim-major layout."""
                W = w_r[li]
                rs = rs_r[li]
                kin = len(x_tiles)
                mout = n_out // P

                ps_mu = psp.tile([1, 512], f32, tag="ps_mu")
                for kt in range(kin):
                    nc.tensor.matmul(ps_mu[:, :T], lhsT=rs[kt][:],
                                     rhs=x_tiles[kt][:, :T],
                                     start=(kt == 0), stop=(kt == kin - 1))
                mu_neg = sbp.tile([1, 512], f32r, tag="mu_neg")
                nc.scalar.activation(mu_neg[:, :T], ps_mu[:, :T], AF.Identity,
                                     bias=mb_sb[:, li:li + 1], scale=-1.0)

                sq, rl = [], []
                for m in range(mout):
                    ps_y = psp_y.tile([P, 512], f32, tag="ps_y")
                    for kt in range(kin):
                        nc.tensor.matmul(ps_y[:, :T],
                                         lhsT=W[kt][:, m * P:(m + 1) * P],
                                         rhs=x_tiles[kt][:, :T],
                                         start=(kt == 0), stop=False)
                    nc.tensor.matmul(ps_y[:, :T], lhsT=ones_row[:],
                                     rhs=mu_neg[:, :T], start=False, stop=True)
                    sq_m = sbp.tile([P, 512], f32r, tag=f"sq{m}")
                    nc.scalar.activation(sq_m[:, :T], ps_y[:, :T], AF.Square)
                    sq.append(sq_m)
                    rl_m = sbp.tile([P, 512], f32r, tag=f"rl{m}")
                    nc.scalar.activation(rl_m[:, :T], ps_y[:, :T],
                                         AF.Relu if relu else AF.Copy)
                    rl.append(rl_m)

                ps_ssq = psp.tile([1, 512], f32, tag="ps_ssq")
                for m in range(mout):
                    nc.tensor.matmul(ps_ssq[:, :T], lhsT=ones_col[:],
                                     rhs=sq[m][:, :T],
                                     start=(m == 0), stop=(m == mout - 1))
                std = sbp.tile([1, 512], f32, tag="std")
                nc.scalar.activation(std[:, :T], ps_ssq[:, :T], AF.Sqrt,
                                     bias=eps_t[:, :1], scale=1.0 / n_out)
                rsig = sbp.tile([1, 512], f32r, tag="rsig")
                with nc.allow_low_precision(reason="f32r rounding of rsig"):
                    nc.vector.reciprocal(rsig[:, :T], std[:, :T])
                ps_rb = psp.tile([P, 512], f32, tag="ps_rb")
                nc.tensor.matmul(ps_rb[:, :T], lhsT=ones_row[:],
                                 rhs=rsig[:, :T], start=True, stop=True)

                outs = []
                for m in range(mout):
                    o_m = sbp.tile([P, 512], f32r, tag=f"xo{m}")
                    nc.vector.tensor_mul(o_m[:, :T], rl[m][:, :T],
                                         ps_rb[:, :T])
                    outs.append(o_m)
                return outs

            # ======================= Phase A =======================
            with (
                tc.tile_pool(name="pa_sb", bufs=2) as pa_sb,
                tc.tile_pool(name="pa_ps", bufs=1, space="PSUM") as pa_ps,
                tc.tile_pool(name="pa_ps_y", bufs=2, space="PSUM") as pa_ps_y,
            ):
                tok = []
                s0 = 0
                while s0 < S:
                    T = min(512, S - s0)
                    tok.append((s0, T))
                    s0 += T

                for (s0, T) in tok:
                    x0 = []
                    for kt in range(KD):
                        f_t = pa_sb.tile([P, 512], f32, tag=f"feat{kt}")
                        nc.sync.dma_start(f_t[:, :T],
                                          featT[kt * P:(kt + 1) * P, s0:s0 + T])
                        e_t = pa_sb.tile([P, 512], f32, tag=f"emb{kt}")
                        nc.sync.dma_start(e_t[:, :T],
                                          embT[kt * P:(kt + 1) * P, s0:s0 + T])
                        x_t = pa_sb.tile([P, 512], f32r, tag=f"x0_{kt}")
                        nc.vector.tensor_add(x_t[:, :T], f_t[:, :T], e_t[:, :T])
                        x0.append(x_t)

                    x1 = layer(pa_sb, pa_ps, pa_ps_y, x0, 0, H, T, relu=True)
                    x2 = layer(pa_sb, pa_ps, pa_ps_y, x1, 1, H, T, relu=True)
                    h2 = layer(pa_sb, pa_ps, pa_ps_y, x2, 2, H, T, relu=True)

                    # z = h2 @ W3 (dim-major, no LN)
                    zsb = []
                    for m in range(KD):
                        ps_z = pa_ps_y.tile([P, 512], f32, tag="ps_y",
                                            name=f"ps_z{m}")
                        for kt in range(KH):
                            nc.tensor.matmul(
                                ps_z[:, :T],
                                lhsT=w_r[3][kt][:, m * P:(m + 1) * P],
                                rhs=h2[kt][:, :T],
                                start=(kt == 0), stop=(kt == KH - 1))
                        z_m = pa_sb.tile([P, 512], f32r, tag=f"zsb{m}")
                        nc.scalar.activation(z_m[:, :T], ps_z[:, :T], AF.Copy)
                        zsb.append(z_m)

                    # s (with b_att folded) and t
                    ps_s = pa_ps.tile([1, 512], f32, tag="ps_st", name="ps_s")
                    for kt in range(KH):
                        nc.tensor.matmul(ps_s[:, :T], lhsT=wattr[kt][:, 0:1],
                                         rhs=h2[kt][:, :T],
                                         start=(kt == 0), stop=(kt == KH - 1))
                    nc.scalar.activation(s_row[:, s0:s0 + T], ps_s[:, :T],
                                         AF.Identity, bias=batt_sb[:, :1])
                    ps_t = pa_ps.tile([1, 512], f32, tag="ps_st", name="ps_t")
                    for kt in range(KH):
                        nc.tensor.matmul(ps_t[:, :T], lhsT=wattr[kt][:, 1:2],
                                         rhs=h2[kt][:, :T],
                                         start=(kt == 0), stop=(kt == KH - 1))
                    t_row = pa_sb.tile([1, 512], f32r, tag="t_row")
                    nc.vector.tensor_copy(t_row[:, :T], ps_t[:, :T])

                    for g in range(T // P):
                        rowt = pa_sb.tile([P, R], bf16, tag="rowt")
                        znt = pa_sb.tile([P, D], f32, tag="znt")
                        for m in range(KD):
                            ps_tr = pa_ps.tile([P, P], f32, tag="ps_tr")
                            nc.tensor.transpose(
                                ps_tr[:].bitcast(f32r),
                                zsb[m][:, g * P:(g + 1) * P], identr[:])
                            nc.scalar.activation(rowt[:, m * P:(m + 1) * P],
                                                 ps_tr[:], AF.Copy)
                            nc.scalar.activation(znt[:, m * P:(m + 1) * P],
                                                 ps_tr[:], AF.Copy, scale=2.0)
                        ps_tc = pa_ps.tile([P, 2], f32, tag="ps_tc")
                        nc.tensor.matmul(ps_tc[:],
                                         lhsT=t_row[:, g * P:(g + 1) * P],
                                         rhs=ones_1[:], start=True, stop=True)
                        nc.scalar.activation(rowt[:, D:D + 1],
                                             ps_tc[:, 0:1], AF.Copy)
                        nc.sync.dma_start(
                            ag_in[s0 + g * P:s0 + (g + 1) * P, :], rowt[:])
                        nc.sync.dma_start(
                            zN_dram[s0 + g * P:s0 + (g + 1) * P, :], znt[:])

            # ================== AllGather (chunked) ==================
            bb, crb = cfg.bucket_base, cfg.chunk_row_base
            for k in range(len(bb) - 1):
                nc.gpsimd.collective_compute(
                    "AllGather", ALU.bypass,
                    replica_groups=[list(range(cfg.n_cores))],
                    ins=[ag_in[bb[k] * P:bb[k + 1] * P, :]],
                    outs=[ag_out[crb[k]:crb[k + 1], :]],
                )

            # ======================= Phase B =======================
            with (
                tc.tile_pool(name="pb_g", bufs=cfg.g_bufs) as pb_g,
                tc.tile_pool(name="pb_sb", bufs=2) as pb_sb,
                tc.tile_pool(name="pb_oat", bufs=2) as pb_oat,
                tc.tile_pool(name="pb_ps", bufs=2, space="PSUM") as pb_ps,
            ):
                ag_lo = ag_out[0:cfg.half_rows, :]
                ag_hi = ag_out[cfg.half_rows:cfg.totn, :]

                # zero gather buffers once: slots beyond the per-bucket
                # actual count keep stale-but-finite data afterwards
                for i in range(cfg.g_bufs):
                    gz = pb_g.tile([P, CPB, R], bf16, tag="G", name=f"Gz{i}")
                    nc.vector.memset(gz[:], 0.0)

                def do_agg(st):
                    (b, G, oat) = st
                    ps_agg = pb_ps.tile([P, D], f32, tag="ps_agg")
                    for ch in range(CPB):
                        nc.tensor.matmul(ps_agg[:],
                                         lhsT=oat[:, ch * P:(ch + 1) * P],
                                         rhs=G[:, ch, 0:D],
                                         start=(ch == 0), stop=(ch == CPB - 1))
                    znt = pb_sb.tile([P, D], f32, tag="zn_in")
                    nc.sync.dma_start(znt[:], zN_dram[b * P:(b + 1) * P, :])
                    y = pb_sb.tile([P, D], f32, tag="y")
                    nc.vector.tensor_add(y[:], ps_agg[:], znt[:])
                    r1 = pb_sb.tile([P, 1], f32, tag="r1")
                    nc.vector.tensor_reduce(r1[:], y[:],
                                            mybir.AxisListType.X, ALU.add)
                    sqd = pb_sb.tile([P, D], f32, tag="sqd")
                    nc.scalar.activation(sqd[:], y[:], AF.Square)
                    r2 = pb_sb.tile([P, 1], f32, tag="r2")
                    nc.vector.tensor_reduce(r2[:], sqd[:],
                                            mybir.AxisListType.X, ALU.add)
                    mu = pb_sb.tile([P, 1], f32, tag="mu")
                    nc.vector.tensor_scalar(out=mu[:], in0=r1[:],
                                            scalar1=1.0 / D, scalar2=None,
                                            op0=ALU.mult)
                    mu2 = pb_sb.tile([P, 1], f32, tag="mu2")
                    nc.vector.tensor_mul(mu2[:], mu[:], mu[:])
                    var = pb_sb.tile([P, 1], f32, tag="var")
                    nc.vector.tensor_scalar(out=var[:], in0=r2[:],
                                            scalar1=1.0 / D, scalar2=mu2[:],
                                            op0=ALU.mult, op1=ALU.subtract)
                    std = pb_sb.tile([P, 1], f32, tag="stdb")
                    nc.scalar.activation(std[:], var[:], AF.Sqrt,
                                         bias=eps_col[:, :1])
                    rsig = pb_sb.tile([P, 1], f32, tag="rsigb")
                    with nc.allow_low_precision(reason="ln rsig"):
                        nc.vector.reciprocal(rsig[:], std[:])
                    mnr = pb_sb.tile([P, 1], f32, tag="mnr")
                    nc.vector.tensor_scalar(out=mnr[:], in0=mu[:],
                                            scalar1=-1.0, scalar2=rsig[:],
                                            op0=ALU.mult, op1=ALU.mult)
                    on = pb_sb.tile([P, D], f32, tag="on")
                    nc.scalar.activation(on[:], y[:], AF.Identity,
                                         bias=mnr[:, :1], scale=rsig[:, :1])
                    nc.sync.dma_start(outN[b * P:(b + 1) * P, :], on[:])

                prev = None
                for b in range(B):
                    G = pb_g.tile([P, CPB, R], bf16, tag="G")
                    for half in range(2):
                        k = cfg.half_slots if cnts is None else \
                            int(cnts[b * 2 + half])
                        nc.gpsimd.dma_gather(
                            out_ap=G[:, half * (CPB // 2):
                                     (half + 1) * (CPB // 2), :],
                            in_ap=(ag_lo if half == 0 else ag_hi),
                            idxs_ap=idx_sb[:, (b * 2 + half) * HS16:
                                           (b * 2 + half + 1) * HS16],
                            num_idxs=cfg.half_slots,
                            num_idxs_reg=k,
                            elem_size=R,
                        )

                    # s broadcast over partitions (same for all chunks)
                    ps_sbc = pb_ps.tile([P, P], f32, tag="ps_sbc")
                    nc.tensor.matmul(ps_sbc[:], lhsT=ones_row_bf[:],
                                     rhs=s_row[0:1, b * P:(b + 1) * P],
                                     start=True, stop=True)

                    # att = sigmoid(s[dst] + t[slot]):  t enters as the
                    # per-partition activation bias (cast to f32 first)
                    tcol = pb_sb.tile([P, CPB], f32, tag="tcol")
                    nc.vector.tensor_copy(tcol[:], G[:, :, D])
                    sg = pb_oat.tile([P, CPB * P], bf16, tag="sg")
                    for ch in range(CPB):
                        nc.scalar.activation(sg[:, ch * P:(ch + 1) * P],
                                             ps_sbc[:], AF.Sigmoid,
                                             bias=tcol[:, ch:ch + 1])

                    oht = pb_sb.tile([P, CPB * P], bf16, tag="oht")
                    nc.sync.dma_start(oht[:],
                                      ohT[:, b * CPB * P:(b + 1) * CPB * P])
                    oat = pb_oat.tile([P, CPB * P], bf16, tag="oat")
                    nc.vector.tensor_mul(oat[:], sg[:], oht[:])

                    if prev is not None:
                        do_agg(prev)
                    prev = (b, G, oat)
                do_agg(prev)

    nc.compile()
    return nc


# ---------------------------------------------------------------------------
# Host-side preparation
# ---------------------------------------------------------------------------

def host_prep(cfg: Cfg, node_ids, edge_index, node_features, emb_table):
    n = node_ids.shape[0]
    S, B, CPB = cfg.shard, cfg.buckets, cfg.cpb
    NCB = cfg.n_cores * B
    row = np.asarray(edge_index[0], np.int64)
    col = np.asarray(edge_index[1], np.int64)
    deg = np.bincount(row, minlength=n)

    order = np.argsort(-deg, kind="stable")
    gb = np.empty(n, np.int64)
    gb[order] = np.arange(n) % NCB

    def slots_for(gb_):
        slot = np.zeros(n, np.int64)
        o2 = np.argsort(gb_, kind="stable")
        gs = gb_[o2]
        start_of = np.searchsorted(gs, np.arange(NCB))
        slot[o2] = np.arange(n) - start_of[gs]
        return slot

    slot_in_b = slots_for(gb)
    assert slot_in_b.max() < P

    bb = np.array(cfg.bucket_base)
    crows = np.array(cfg.chunk_rows)
    crb = np.array(cfg.chunk_row_base)

    def gidx_of(core, s):
        bkt = s // P
        k = np.searchsorted(bb, bkt, side="right") - 1
        return crb[k] + core * crows[k] + (s - bb[k] * P)

    lim = cfg.half_slots
    for it in range(500):
        gsl = gidx_of(gb // B, (gb % B) * P + slot_in_b)
        src_half = (gsl >= cfg.half_rows).astype(np.int64)[col]
        loads = np.zeros((NCB, 2), np.int64)
        np.add.at(loads, (gb[row], src_half), 1)
        over = np.argwhere(loads > lim)
        if len(over) == 0:
            break
        ob, ohalf = over[np.argmax(loads[over[:, 0], over[:, 1]])]
        core = ob // B
        cand_b = np.arange(core * B, (core + 1) * B)
        bn = np.bincount(gb, minlength=NCB)
        mask_e = (gb[row] == ob) & (src_half == ohalf)
        contrib = np.bincount(row[mask_e], minlength=n)
        nodes_in_ob = np.where(gb == ob)[0]
        v = nodes_in_ob[np.argmax(contrib[nodes_in_ob])]
        room = bn[cand_b] < P
        scores = loads[cand_b].max(1).astype(np.float64)
        scores[~room] = np.inf
        scores[cand_b == ob] = np.inf
        tb = cand_b[np.argmin(scores)]
        if not np.isfinite(scores.min()):
            raise RuntimeError("bucket fix-up failed: no room")
        gb[v] = tb
        slot_in_b = slots_for(gb)
    else:
        raise RuntimeError("bucket fix-up did not converge")

    gsl = gidx_of(gb // B, (gb % B) * P + slot_in_b)

    perm = np.full((cfg.n_cores, S), -1, np.int64)
    perm[gb // B, (gb % B) * P + slot_in_b] = np.arange(n)

    e_core = gb[row] // B
    e_b = gb[row] % B
    e_d = slot_in_b[row]
    e_half = (gsl[col] >= cfg.half_rows).astype(np.int64)
    e_gidx = gsl[col] - e_half * cfg.half_rows

    HS16 = cfg.half_slots // 16
    dg_all = np.zeros((cfg.n_cores, P, B * 2 * HS16), np.int16)
    oh_all = np.zeros((cfg.n_cores, P, B * CPB * P), np.uint16)
    cnt_all = np.zeros((cfg.n_cores, B * 2), np.int64)
    ONE_BF16 = np.uint16(0x3F80)

    key = ((e_core * B + e_b) * 2 + e_half)
    eo = np.argsort(key, kind="stable")
    ks = key[eo]
    bounds = np.searchsorted(ks, np.arange(NCB * 2 + 1))
    for c in range(cfg.n_cores):
        for b in range(B):
            for half in range(2):
                kk = (c * B + b) * 2 + half
                cnt_all[c, b * 2 + half] = bounds[kk + 1] - bounds[kk]
    # num_idxs_reg must equal count_nonzero(idx >= 0) on EVERY core (SPMD),
    # so pad each (bucket, half) idx list with dummy valid 0-indices up to
    # the cross-core max, with -1 sentinels beyond.
    cnts_max = cnt_all.max(axis=0)
    for c in range(cfg.n_cores):
        for b in range(B):
            for half in range(2):
                kk = (c * B + b) * 2 + half
                sel = eo[bounds[kk]:bounds[kk + 1]]
                k = len(sel)
                kp = int(cnts_max[b * 2 + half])
                assert k <= kp <= cfg.half_slots, (c, b, half, k, kp)
                idx_pad = np.full(cfg.half_slots, -1, np.int64)
                idx_pad[:k] = e_gidx[sel]
                idx_pad[k:kp] = 0
                blk = idx_pad.reshape(HS16, 16).T.astype(np.int16)
                off = (b * 2 + half) * HS16
                dg_all[c, :, off:off + HS16] = np.tile(blk, (8, 1))
                # one-hot: slot j of this half -> (p=j%128, ch=half*5+j//128)
                j = np.arange(k)
                pp = j % P
                ch = half * (CPB // 2) + j // P
                dd = e_d[sel]
                oh_all[c, pp, (b * CPB + ch) * P + dd] = ONE_BF16

    featT_all = np.zeros((cfg.n_cores, cfg.d_in, S), np.float32)
    embT_all = np.zeros((cfg.n_cores, cfg.d_in, S), np.float32)
    nf = np.asarray(node_features, np.float32)
    er = np.asarray(emb_table, np.float32)[np.asarray(node_ids, np.int64)]
    for c in range(cfg.n_cores):
        pc = perm[c]
        valid = pc >= 0
        featT_all[c][:, valid] = nf[pc[valid]].T
        embT_all[c][:, valid] = er[pc[valid]].T

    return perm, featT_all, embT_all, dg_all, oh_all, cnt_all


_BUILD_CACHE = {}


def _get_nc(cfg: Cfg, cnts=None):
    key = (cfg, None if cnts is None else tuple(int(x) for x in cnts))
    if key not in _BUILD_CACHE:
        _BUILD_CACHE[key] = build(cfg, cnts)
    return _BUILD_CACHE[key]


def run(cfg: Cfg, node_ids, edge_index, node_features, emb_table,
        W0, b0, g0, be0, W1, b1, g1, be1, W2, b2, g2, be2,
        W3, b3, g3, be3, w_att, b_att):
    import ml_dtypes
    D, H = cfg.d_in, cfg.d_hid
    b_list = [np.asarray(x, np.float32) for x in (b0, b1, b2, b3)]
    g_list = [np.asarray(x, np.float32) for x in (g0, g1, g2, g3)]
    be_list = [np.asarray(x, np.float32) for x in (be0, be1, be2, be3)]
    with_b = any(np.any(x != 0) for x in b_list)
    with_gbe = (any(np.any(x != 1) for x in g_list)
                or any(np.any(x != 0) for x in be_list))
    assert not (with_b or with_gbe), \
        "v2 kernel fast path requires default b/g/be params"
    cfg = replace(cfg, b_att=float(np.asarray(b_att)))

    perm, featT_all, embT_all, dg_all, oh_all, cnt_all = host_prep(
        cfg, node_ids, edge_index, node_features, emb_table)
    cnts_max = cnt_all.max(axis=0)

    W = [np.asarray(x, np.float32) for x in (W0, W1, W2, W3)]
    rs = [(w.sum(1) / H).astype(np.float32)[:, None] for w in W[:3]]
    wa = np.asarray(w_att, np.float32)
    watt2 = np.stack([wa[:H], wa[H:]], axis=1)

    mbias = np.zeros((1, 8), np.float32)
    for i, x in enumerate(b_list):
        mbias[0, i] = float(x.mean())

    nc = _get_nc(cfg, cnts_max)
    in_maps = []
    for c in range(cfg.n_cores):
        in_maps.append(dict(
            featT=featT_all[c], embT=embT_all[c],
            w0=W[0], w1=W[1], w2=W[2], w3=W[3], watt=watt2,
            rs0=rs[0], rs1=rs[1], rs2=rs[2],
            dgidx=dg_all[c],
            ohT=oh_all[c].view(ml_dtypes.bfloat16),
            mbias=mbias,
        ))
    res = run_bass_kernel_spmd(nc, in_maps, core_ids=list(range(cfg.n_cores)),
                               trace=cfg.trace)
    n = node_ids.shape[0]
    out = np.zeros((n, D), np.float32)
    for c in range(cfg.n_cores):
        pc = perm[c]
        valid = pc >= 0
        out[pc[valid]] = res.results[c]["outN"][valid]
    return out, res


def kernel(**inputs) -> np.ndarray:
    out, _ = run(CFG, **inputs)
    return out


# revision 30
# speedup vs baseline: 1.0495x; 1.0495x over previous
"""Trainium2 Bass kernel for MemoryEfficientISNE GNN message passing (v2).

Full inputs in, full output out. 8-way data-parallel over nodes with a
balanced node->(core, bucket, slot) permutation computed on the host.

v2 design ("z-space" aggregation):
  - Phase A: 3-layer MLP in dim-major layout (fp32r matmuls, matmul-based
    LayerNorm stats), then z = h2 @ W3 (no LN), s = h2@wr + b_att, t = h2@wc.
    Emits per-node 768B rows [z bf16 x256 | t bf16 | pad] to ag_in, and
    2*z node-major f32 to zN_dram.
  - Chunked AllGather (4 chunks on bucket-group boundaries) of the rows,
    overlapping the phase A tail.
  - Phase B: per destination bucket, dma_gather 2x640 edge rows (lo/hi
    halves). Attention: one PE matmul broadcasts s over partitions; ten
    sigmoid activations add the per-edge t as per-partition bias; a host
    precomputed one-hot (dest routing) is DMA'd and multiplied in; the
    aggregation is 10 accumulating [slot,dst]^T x [slot,z] matmuls into a
    [dst, 256] PSUM tile. Final LayerNorm runs node-major on vector/scalar
    (LN(2*z + agg) == LN(z + 0.5*agg) by LN scale invariance).

Self-contained: hardcodes the problem shapes from the task spec.
"""
from dataclasses import dataclass, replace

import numpy as np

import concourse.bacc as bacc
import concourse.bass as bass
import concourse.tile as tile
from concourse import mybir
from concourse.bass_utils import run_bass_kernel_spmd
from concourse.masks import make_identity

f32 = mybir.dt.float32
f32r = mybir.dt.float32r
bf16 = mybir.dt.bfloat16
i16 = mybir.dt.int16
AF = mybir.ActivationFunctionType
ALU = mybir.AluOpType

LN_EPS = 1e-5
P = 128


@dataclass(frozen=True)
class Cfg:
    n_cores: int = 8
    d_in: int = 256          # D
    d_hid: int = 512         # H
    shard: int = 6272        # padded nodes per core (= buckets * 128)
    cpb: int = 10            # gather chunks per bucket (5 lo + 5 hi)
    row: int = 384           # ag row in bf16 elems (768B, %256==0)
    g_bufs: int = 6          # gather destination buffers in SBUF
    trace: bool = False
    b_att: float = 0.0

    @property
    def buckets(self):
        return self.shard // P

    @property
    def totn(self):
        return self.n_cores * self.shard

    @property
    def half_slots(self):
        return (self.cpb // 2) * P

    n_ag_chunks: int = 1

    # AllGather chunking: global row of (core c, slot s in chunk k) =
    # chunk_row_base[k] + c*chunk_rows[k] + (s - 128*bucket_base[k]).
    @property
    def bucket_base(self):
        if self.n_ag_chunks == 1:
            return [0, self.buckets]
        if self.n_ag_chunks == 2:
            return [0, 24, self.buckets]
        return [0, 12, 24, 36, self.buckets]

    @property
    def chunk_rows(self):
        bb = self.bucket_base
        return [(bb[k + 1] - bb[k]) * P for k in range(len(bb) - 1)]

    @property
    def chunk_row_base(self):
        out = [0]
        for r in self.chunk_rows:
            out.append(out[-1] + r * self.n_cores)
        return out

    @property
    def half_rows(self):
        # both halves must stay < 32768 rows for int16 gather indices
        if self.n_ag_chunks == 1:
            return self.totn // 2
        return self.chunk_row_base[len(self.bucket_base) // 2]


CFG = Cfg()


def build(cfg: Cfg, cnts=None):
    nc = bacc.Bacc("TRN2", target_bir_lowering=False, debug=False,
                   num_devices=cfg.n_cores)
    D, H, S = cfg.d_in, cfg.d_hid, cfg.shard
    B, CPB, R = cfg.buckets, cfg.cpb, cfg.row
    KD, KH = D // P, H // P
    HS16 = cfg.half_slots // 16

    # ---- I/O ----
    featT = nc.dram_tensor("featT", [D, S], f32, kind="ExternalInput").ap()
    embT = nc.dram_tensor("embT", [D, S], f32, kind="ExternalInput").ap()
    w_in = [nc.dram_tensor(f"w{i}", shp, f32, kind="ExternalInput").ap()
            for i, shp in enumerate([[D, H], [H, H], [H, H], [H, D]])]
    watt = nc.dram_tensor("watt", [H, 2], f32, kind="ExternalInput").ap()
    rs_in = [nc.dram_tensor(f"rs{i}", [[D, 1], [H, 1], [H, 1]][i], f32,
                            kind="ExternalInput").ap() for i in range(3)]
    dgidx = nc.dram_tensor("dgidx", [P, B * 2 * HS16], i16,
                           kind="ExternalInput").ap()
    ohT = nc.dram_tensor("ohT", [P, B * CPB * P], bf16,
                         kind="ExternalInput").ap()
    mbias = nc.dram_tensor("mbias", [1, 8], f32, kind="ExternalInput").ap()

    outN = nc.dram_tensor("outN", [S, D], f32, kind="ExternalOutput").ap()

    # ---- internal DRAM ----
    ag_in = nc.dram_tensor("ag_in", [S, R], bf16, kind="Internal").ap()
    ag_out = nc.dram_tensor(
        "ag_out", [cfg.totn, R], bf16, kind="Internal",
        addr_space="Shared" if cfg.n_cores > 4 else "Local").ap()
    zN_dram = nc.dram_tensor("zN_dram", [S, D], f32, kind="Internal").ap()

    with tile.TileContext(nc) as tc:
        with (
            tc.tile_pool(name="consts", bufs=1) as consts,
            tc.tile_pool(name="wstage", bufs=2) as wstage,
        ):
            def load_r(src_ap, shape, name):
                t_f = wstage.tile([P, 512], f32, tag="wstage")
                nc.sync.dma_start(t_f[:shape[0], :shape[1]], src_ap)
                t_r = consts.tile(shape, f32r, tag=name)
                nc.vector.tensor_copy(t_r[:], t_f[:shape[0], :shape[1]])
                return t_r

            w_r = []
            for i, w in enumerate(w_in):
                kin = w.shape[0] // P
                w_r.append([load_r(w[kt * P:(kt + 1) * P, :], [P, w.shape[1]],
                                   f"w{i}_{kt}") for kt in range(kin)])
            wattr = [load_r(watt[kt * P:(kt + 1) * P, :], [P, 2], f"watt_{kt}")
                     for kt in range(KH)]
            rs_r = []
            for i, rs in enumerate(rs_in):
                kin = rs.shape[0] // P
                rs_r.append([load_r(rs[kt * P:(kt + 1) * P, :], [P, 1],
                                    f"rs{i}_{kt}") for kt in range(kin)])

            ones_f = consts.tile([P, P], f32, tag="ones_f")
            nc.vector.memset(ones_f[:], 1.0)
            ones_row = consts.tile([1, P], f32r, tag="ones_row")
            nc.vector.tensor_copy(ones_row[:], ones_f[:1, :])
            ones_row_bf = consts.tile([1, P], bf16, tag="ones_row_bf")
            nc.vector.tensor_copy(ones_row_bf[:], ones_f[:1, :])
            ones_1 = consts.tile([1, 2], f32r, tag="ones_1")
            nc.vector.tensor_copy(ones_1[:], ones_f[:1, :2])
            ones_col = consts.tile([P, 1], f32r, tag="ones_col")
            nc.vector.tensor_copy(ones_col[:], ones_f[:, :1])

            ident_f = consts.tile([P, P], f32, tag="ident_f")
            make_identity(nc, ident_f[:])
            identr = consts.tile([P, P], f32r, tag="identr")
            nc.vector.tensor_copy(identr[:], ident_f[:])

            idx_sb = consts.tile([P, B * 2 * HS16], i16, tag="idx_sb")
            nc.sync.dma_start(idx_sb[:], dgidx)

            mb_sb = consts.tile([1, 8], f32, tag="mb_sb")
            nc.sync.dma_start(mb_sb[:], mbias)
            eps_t = consts.tile([1, 1], f32, tag="eps_t")
            nc.vector.memset(eps_t[:], LN_EPS)
            eps_col = consts.tile([P, 1], f32, tag="eps_col")
            nc.vector.memset(eps_col[:], LN_EPS)
            batt_sb = consts.tile([1, 1], f32, tag="batt_sb")
            nc.vector.memset(batt_sb[:], float(cfg.b_att))

            s_row = consts.tile([1, S], bf16, tag="s_row")

            # --------------------------------------------------------------
            def layer(sbp, psp, psp_y, x_tiles, li, n_out, T, relu):
                """One W@x + LayerNorm (+relu) in dim-major layout."""
                W = w_r[li]
                rs = rs_r[li]
                kin = len(x_tiles)
                mout = n_out // P

                ps_mu = psp.tile([1, 512], f32, tag="ps_mu")
                for kt in range(kin):
                    nc.tensor.matmul(ps_mu[:, :T], lhsT=rs[kt][:],
                                     rhs=x_tiles[kt][:, :T],
                                     start=(kt == 0), stop=(kt == kin - 1))
                mu_neg = sbp.tile([1, 512], f32r, tag="mu_neg")
                nc.scalar.activation(mu_neg[:, :T], ps_mu[:, :T], AF.Identity,
                                     bias=mb_sb[:, li:li + 1], scale=-1.0)

                sq, rl = [], []
                for m in range(mout):
                    ps_y = psp_y.tile([P, 512], f32, tag="ps_y")
                    for kt in range(kin):
                        nc.tensor.matmul(ps_y[:, :T],
                                         lhsT=W[kt][:, m * P:(m + 1) * P],
                                         rhs=x_tiles[kt][:, :T],
                                         start=(kt == 0), stop=False)
                    nc.tensor.matmul(ps_y[:, :T], lhsT=ones_row[:],
                                     rhs=mu_neg[:, :T], start=False, stop=True)
                    sq_m = sbp.tile([P, 512], f32r, tag=f"sq{m}")
                    nc.scalar.activation(sq_m[:, :T], ps_y[:, :T], AF.Square)
                    sq.append(sq_m)
                    rl_m = sbp.tile([P, 512], f32r, tag=f"rl{m}")
                    nc.scalar.activation(rl_m[:, :T], ps_y[:, :T],
                                         AF.Relu if relu else AF.Copy)
                    rl.append(rl_m)

                ps_ssq = psp.tile([1, 512], f32, tag="ps_ssq")
                for m in range(mout):
                    nc.tensor.matmul(ps_ssq[:, :T], lhsT=ones_col[:],
                                     rhs=sq[m][:, :T],
                                     start=(m == 0), stop=(m == mout - 1))
                std = sbp.tile([1, 512], f32, tag="std")
                nc.scalar.activation(std[:, :T], ps_ssq[:, :T], AF.Sqrt,
                                     bias=eps_t[:, :1], scale=1.0 / n_out)
                rsig = sbp.tile([1, 512], f32r, tag="rsig")
                with nc.allow_low_precision(reason="f32r rounding of rsig"):
                    nc.vector.reciprocal(rsig[:, :T], std[:, :T])
                ps_rb = psp.tile([P, 512], f32, tag="ps_rb")
                nc.tensor.matmul(ps_rb[:, :T], lhsT=ones_row[:],
                                 rhs=rsig[:, :T], start=True, stop=True)

                outs = []
                for m in range(mout):
                    o_m = sbp.tile([P, 512], f32r, tag=f"xo{m}")
                    nc.vector.tensor_mul(o_m[:, :T], rl[m][:, :T],
                                         ps_rb[:, :T])
                    outs.append(o_m)
                return outs

            # ======================= Phase A =======================
            with (
                tc.tile_pool(name="pa_sb", bufs=2) as pa_sb,
                tc.tile_pool(name="pa_ps", bufs=1, space="PSUM") as pa_ps,
                tc.tile_pool(name="pa_ps_y", bufs=2, space="PSUM") as pa_ps_y,
            ):
                tok = []
                s0 = 0
                while s0 < S:
                    T = min(512, S - s0)
                    tok.append((s0, T))
                    s0 += T

                for (s0, T) in tok:
                    x0 = []
                    for kt in range(KD):
                        f_t = pa_sb.tile([P, 512], f32, tag=f"feat{kt}")
                        nc.sync.dma_start(f_t[:, :T],
                                          featT[kt * P:(kt + 1) * P, s0:s0 + T])
                        e_t = pa_sb.tile([P, 512], f32, tag=f"emb{kt}")
                        nc.sync.dma_start(e_t[:, :T],
                                          embT[kt * P:(kt + 1) * P, s0:s0 + T])
                        x_t = pa_sb.tile([P, 512], f32r, tag=f"x0_{kt}")
                        nc.vector.tensor_add(x_t[:, :T], f_t[:, :T], e_t[:, :T])
                        x0.append(x_t)

                    x1 = layer(pa_sb, pa_ps, pa_ps_y, x0, 0, H, T, relu=True)
                    x2 = layer(pa_sb, pa_ps, pa_ps_y, x1, 1, H, T, relu=True)
                    h2 = layer(pa_sb, pa_ps, pa_ps_y, x2, 2, H, T, relu=True)

                    # z = h2 @ W3 (dim-major, no LN)
                    zsb = []
                    for m in range(KD):
                        ps_z = pa_ps_y.tile([P, 512], f32, tag="ps_y",
                                            name=f"ps_z{m}")
                        for kt in range(KH):
                            nc.tensor.matmul(
                                ps_z[:, :T],
                                lhsT=w_r[3][kt][:, m * P:(m + 1) * P],
                                rhs=h2[kt][:, :T],
                                start=(kt == 0), stop=(kt == KH - 1))
                        z_m = pa_sb.tile([P, 512], f32r, tag=f"zsb{m}")
                        nc.scalar.activation(z_m[:, :T], ps_z[:, :T], AF.Copy)
                        zsb.append(z_m)

                    # s (with b_att folded) and t
                    ps_s = pa_ps.tile([1, 512], f32, tag="ps_st", name="ps_s")
                    for kt in range(KH):
                        nc.tensor.matmul(ps_s[:, :T], lhsT=wattr[kt][:, 0:1],
                                         rhs=h2[kt][:, :T],
                                         start=(kt == 0), stop=(kt == KH - 1))
                    nc.scalar.activation(s_row[:, s0:s0 + T], ps_s[:, :T],
                                         AF.Identity, bias=batt_sb[:, :1])
                    ps_t = pa_ps.tile([1, 512], f32, tag="ps_st", name="ps_t")
                    for kt in range(KH):
                        nc.tensor.matmul(ps_t[:, :T], lhsT=wattr[kt][:, 1:2],
                                         rhs=h2[kt][:, :T],
                                         start=(kt == 0), stop=(kt == KH - 1))
                    t_row = pa_sb.tile([1, 512], f32r, tag="t_row")
                    nc.vector.tensor_copy(t_row[:, :T], ps_t[:, :T])

                    for g in range(T // P):
                        rowt = pa_sb.tile([P, R], bf16, tag="rowt")
                        znt = pa_sb.tile([P, D], f32, tag="znt")
                        for m in range(KD):
                            ps_tr = pa_ps.tile([P, P], f32, tag="ps_tr")
                            nc.tensor.transpose(
                                ps_tr[:].bitcast(f32r),
                                zsb[m][:, g * P:(g + 1) * P], identr[:])
                            nc.scalar.activation(rowt[:, m * P:(m + 1) * P],
                                                 ps_tr[:], AF.Copy)
                            nc.scalar.activation(znt[:, m * P:(m + 1) * P],
                                                 ps_tr[:], AF.Copy, scale=2.0)
                        ps_tc = pa_ps.tile([P, 2], f32, tag="ps_tc")
                        nc.tensor.matmul(ps_tc[:],
                                         lhsT=t_row[:, g * P:(g + 1) * P],
                                         rhs=ones_1[:], start=True, stop=True)
                        nc.scalar.activation(rowt[:, D:D + 1],
                                             ps_tc[:, 0:1], AF.Copy)
                        nc.sync.dma_start(
                            ag_in[s0 + g * P:s0 + (g + 1) * P, :], rowt[:])
                        nc.sync.dma_start(
                            zN_dram[s0 + g * P:s0 + (g + 1) * P, :], znt[:])

            # ================== AllGather (chunked) ==================
            bb, crb = cfg.bucket_base, cfg.chunk_row_base
            for k in range(len(bb) - 1):
                nc.gpsimd.collective_compute(
                    "AllGather", ALU.bypass,
                    replica_groups=[list(range(cfg.n_cores))],
                    ins=[ag_in[bb[k] * P:bb[k + 1] * P, :]],
                    outs=[ag_out[crb[k]:crb[k + 1], :]],
                )

            # ======================= Phase B =======================
            with (
                tc.tile_pool(name="pb_g", bufs=cfg.g_bufs) as pb_g,
                tc.tile_pool(name="pb_sb", bufs=2) as pb_sb,
                tc.tile_pool(name="pb_oat", bufs=2) as pb_oat,
                tc.tile_pool(name="pb_ps", bufs=2, space="PSUM") as pb_ps,
            ):
                ag_lo = ag_out[0:cfg.half_rows, :]
                ag_hi = ag_out[cfg.half_rows:cfg.totn, :]

                # zero gather buffers once: slots beyond the per-bucket
                # actual count keep stale-but-finite data afterwards
                for i in range(cfg.g_bufs):
                    gz = pb_g.tile([P, CPB, R], bf16, tag="G", name=f"Gz{i}")
                    nc.vector.memset(gz[:], 0.0)

                def do_agg(st):
                    (b, G, oat) = st
                    ps_agg = pb_ps.tile([P, D], f32, tag="ps_agg")
                    for ch in range(CPB):
                        nc.tensor.matmul(ps_agg[:],
                                         lhsT=oat[:, ch * P:(ch + 1) * P],
                                         rhs=G[:, ch, 0:D],
                                         start=(ch == 0), stop=(ch == CPB - 1))
                    znt = pb_sb.tile([P, D], f32, tag="zn_in")
                    nc.sync.dma_start(znt[:], zN_dram[b * P:(b + 1) * P, :])
                    y = pb_sb.tile([P, D], f32, tag="y")
                    nc.vector.tensor_add(y[:], ps_agg[:], znt[:])
                    r1 = pb_sb.tile([P, 1], f32, tag="r1")
                    nc.vector.tensor_reduce(r1[:], y[:],
                                            mybir.AxisListType.X, ALU.add)
                    sqd = pb_sb.tile([P, D], f32, tag="sqd")
                    nc.scalar.activation(sqd[:], y[:], AF.Square)
                    r2 = pb_sb.tile([P, 1], f32, tag="r2")
                    nc.vector.tensor_reduce(r2[:], sqd[:],
                                            mybir.AxisListType.X, ALU.add)
                    mu = pb_sb.tile([P, 1], f32, tag="mu")
                    nc.vector.tensor_scalar(out=mu[:], in0=r1[:],
                                            scalar1=1.0 / D, scalar2=None,
                                            op0=ALU.mult)
                    mu2 = pb_sb.tile([P, 1], f32, tag="mu2")
                    nc.vector.tensor_mul(mu2[:], mu[:], mu[:])
                    var = pb_sb.tile([P, 1], f32, tag="var")
                    nc.vector.tensor_scalar(out=var[:], in0=r2[:],
                                            scalar1=1.0 / D, scalar2=mu2[:],
                                            op0=ALU.mult, op1=ALU.subtract)
                    std = pb_sb.tile([P, 1], f32, tag="stdb")
                    nc.scalar.activation(std[:], var[:], AF.Sqrt,
                                         bias=eps_col[:, :1])
                    rsig = pb_sb.tile([P, 1], f32, tag="rsigb")
                    with nc.allow_low_precision(reason="ln rsig"):
                        nc.vector.reciprocal(rsig[:], std[:])
                    mnr = pb_sb.tile([P, 1], f32, tag="mnr")
                    nc.vector.tensor_scalar(out=mnr[:], in0=mu[:],
                                            scalar1=-1.0, scalar2=rsig[:],
                                            op0=ALU.mult, op1=ALU.mult)
                    on = pb_sb.tile([P, D], f32, tag="on")
                    nc.scalar.activation(on[:], y[:], AF.Identity,
                                         bias=mnr[:, :1], scale=rsig[:, :1])
                    nc.sync.dma_start(outN[b * P:(b + 1) * P, :], on[:])

                prev = None
                for b in range(B):
                    G = pb_g.tile([P, CPB, R], bf16, tag="G")
                    for half in range(2):
                        k = cfg.half_slots if cnts is None else \
                            int(cnts[b * 2 + half])
                        nc.gpsimd.dma_gather(
                            out_ap=G[:, half * (CPB // 2):
                                     (half + 1) * (CPB // 2), :],
                            in_ap=(ag_lo if half == 0 else ag_hi),
                            idxs_ap=idx_sb[:, (b * 2 + half) * HS16:
                                           (b * 2 + half + 1) * HS16],
                            num_idxs=cfg.half_slots,
                            num_idxs_reg=k,
                            elem_size=R,
                        )

                    # s broadcast over partitions (same for all chunks)
                    ps_sbc = pb_ps.tile([P, P], f32, tag="ps_sbc")
                    nc.tensor.matmul(ps_sbc[:], lhsT=ones_row_bf[:],
                                     rhs=s_row[0:1, b * P:(b + 1) * P],
                                     start=True, stop=True)

                    # att = sigmoid(s[dst] + t[slot]):  t enters as the
                    # per-partition activation bias (cast to f32 first)
                    tcol = pb_sb.tile([P, CPB], f32, tag="tcol")
                    nc.vector.tensor_copy(tcol[:], G[:, :, D])
                    sg = pb_oat.tile([P, CPB * P], bf16, tag="sg")
                    for ch in range(CPB):
                        nc.scalar.activation(sg[:, ch * P:(ch + 1) * P],
                                             ps_sbc[:], AF.Sigmoid,
                                             bias=tcol[:, ch:ch + 1])

                    oht = pb_sb.tile([P, CPB * P], bf16, tag="oht")
                    nc.sync.dma_start(oht[:],
                                      ohT[:, b * CPB * P:(b + 1) * CPB * P])
                    oat = pb_oat.tile([P, CPB * P], bf16, tag="oat")
                    nc.vector.tensor_mul(oat[:], sg[:], oht[:])

                    if prev is not None:
                        do_agg(prev)
                    prev = (b, G, oat)
                do_agg(prev)

    nc.compile()
    return nc


# ---------------------------------------------------------------------------
# Host-side preparation
# ---------------------------------------------------------------------------

def host_prep(cfg: Cfg, node_ids, edge_index, node_features, emb_table):
    n = node_ids.shape[0]
    S, B, CPB = cfg.shard, cfg.buckets, cfg.cpb
    NCB = cfg.n_cores * B
    row = np.asarray(edge_index[0], np.int64)
    col = np.asarray(edge_index[1], np.int64)
    deg = np.bincount(row, minlength=n)

    order = np.argsort(-deg, kind="stable")
    gb = np.empty(n, np.int64)
    gb[order] = np.arange(n) % NCB

    def slots_for(gb_):
        slot = np.zeros(n, np.int64)
        o2 = np.argsort(gb_, kind="stable")
        gs = gb_[o2]
        start_of = np.searchsorted(gs, np.arange(NCB))
        slot[o2] = np.arange(n) - start_of[gs]
        return slot

    slot_in_b = slots_for(gb)
    assert slot_in_b.max() < P

    bb = np.array(cfg.bucket_base)
    crows = np.array(cfg.chunk_rows)
    crb = np.array(cfg.chunk_row_base)

    def gidx_of(core, s):
        bkt = s // P
        k = np.searchsorted(bb, bkt, side="right") - 1
        return crb[k] + core * crows[k] + (s - bb[k] * P)

    lim = cfg.half_slots
    for it in range(500):
        gsl = gidx_of(gb // B, (gb % B) * P + slot_in_b)
        src_half = (gsl >= cfg.half_rows).astype(np.int64)[col]
        loads = np.zeros((NCB, 2), np.int64)
        np.add.at(loads, (gb[row], src_half), 1)
        over = np.argwhere(loads > lim)
        if len(over) == 0:
            break
        ob, ohalf = over[np.argmax(loads[over[:, 0], over[:, 1]])]
        core = ob // B
        cand_b = np.arange(core * B, (core + 1) * B)
        bn = np.bincount(gb, minlength=NCB)
        mask_e = (gb[row] == ob) & (src_half == ohalf)
        contrib = np.bincount(row[mask_e], minlength=n)
        nodes_in_ob = np.where(gb == ob)[0]
        v = nodes_in_ob[np.argmax(contrib[nodes_in_ob])]
        room = bn[cand_b] < P
        scores = loads[cand_b].max(1).astype(np.float64)
        scores[~room] = np.inf
        scores[cand_b == ob] = np.inf
        tb = cand_b[np.argmin(scores)]
        if not np.isfinite(scores.min()):
            raise RuntimeError("bucket fix-up failed: no room")
        gb[v] = tb
        slot_in_b = slots_for(gb)
    else:
        raise RuntimeError("bucket fix-up did not converge")

    gsl = gidx_of(gb // B, (gb % B) * P + slot_in_b)

    perm = np.full((cfg.n_cores, S), -1, np.int64)
    perm[gb // B, (gb % B) * P + slot_in_b] = np.arange(n)

    e_core = gb[row] // B
    e_b = gb[row] % B
    e_d = slot_in_b[row]
    e_half = (gsl[col] >= cfg.half_rows).astype(np.int64)
    e_gidx = gsl[col] - e_half * cfg.half_rows

    HS16 = cfg.half_slots // 16
    dg_all = np.zeros((cfg.n_cores, P, B * 2 * HS16), np.int16)
    oh_all = np.zeros((cfg.n_cores, P, B * CPB * P), np.uint16)
    cnt_all = np.zeros((cfg.n_cores, B * 2), np.int64)
    ONE_BF16 = np.uint16(0x3F80)

    key = ((e_core * B + e_b) * 2 + e_half)
    eo = np.argsort(key, kind="stable")
    ks = key[eo]
    bounds = np.searchsorted(ks, np.arange(NCB * 2 + 1))
    for c in range(cfg.n_cores):
        for b in range(B):
            for half in range(2):
                kk = (c * B + b) * 2 + half
                cnt_all[c, b * 2 + half] = bounds[kk + 1] - bounds[kk]
    # num_idxs_reg must equal count_nonzero(idx >= 0) on EVERY core (SPMD),
    # so pad each (bucket, half) idx list with dummy valid 0-indices up to
    # the cross-core max, with -1 sentinels beyond.
    cnts_max = cnt_all.max(axis=0)
    for c in range(cfg.n_cores):
        for b in range(B):
            for half in range(2):
                kk = (c * B + b) * 2 + half
                sel = eo[bounds[kk]:bounds[kk + 1]]
                k = len(sel)
                kp = cfg.half_slots
                assert k <= kp, (c, b, half, k)
                idx_pad = np.full(cfg.half_slots, -1, np.int64)
                idx_pad[:k] = e_gidx[sel]
                idx_pad[k:kp] = 0
                blk = idx_pad.reshape(HS16, 16).T.astype(np.int16)
                off = (b * 2 + half) * HS16
                dg_all[c, :, off:off + HS16] = np.tile(blk, (8, 1))
                # one-hot: slot j of this half -> (p=j%128, ch=half*5+j//128)
                j = np.arange(k)
                pp = j % P
                ch = half * (CPB // 2) + j // P
                dd = e_d[sel]
                oh_all[c, pp, (b * CPB + ch) * P + dd] = ONE_BF16

    featT_all = np.zeros((cfg.n_cores, cfg.d_in, S), np.float32)
    embT_all = np.zeros((cfg.n_cores, cfg.d_in, S), np.float32)
    nf = np.asarray(node_features, np.float32)
    er = np.asarray(emb_table, np.float32)[np.asarray(node_ids, np.int64)]
    for c in range(cfg.n_cores):
        pc = perm[c]
        valid = pc >= 0
        featT_all[c][:, valid] = nf[pc[valid]].T
        embT_all[c][:, valid] = er[pc[valid]].T

    return perm, featT_all, embT_all, dg_all, oh_all, cnt_all


_BUILD_CACHE = {}


def _get_nc(cfg: Cfg, cnts=None):
    key = (cfg, None if cnts is None else tuple(int(x) for x in cnts))
    if key not in _BUILD_CACHE:
        _BUILD_CACHE[key] = build(cfg, cnts)
    return _BUILD_CACHE[key]


def run(cfg: Cfg, node_ids, edge_index, node_features, emb_table,
        W0, b0, g0, be0, W1, b1, g1, be1, W2, b2, g2, be2,
        W3, b3, g3, be3, w_att, b_att):
    import ml_dtypes
    D, H = cfg.d_in, cfg.d_hid
    b_list = [np.asarray(x, np.float32) for x in (b0, b1, b2, b3)]
    g_list = [np.asarray(x, np.float32) for x in (g0, g1, g2, g3)]
    be_list = [np.asarray(x, np.float32) for x in (be0, be1, be2, be3)]
    with_b = any(np.any(x != 0) for x in b_list)
    with_gbe = (any(np.any(x != 1) for x in g_list)
                or any(np.any(x != 0) for x in be_list))
    assert not (with_b or with_gbe), \
        "v2 kernel fast path requires default b/g/be params"
    cfg = replace(cfg, b_att=float(np.asarray(b_att)))

    perm, featT_all, embT_all, dg_all, oh_all, cnt_all = host_prep(
        cfg, node_ids, edge_index, node_features, emb_table)
    cnts_max = cnt_all.max(axis=0)

    W = [np.asarray(x, np.float32) for x in (W0, W1, W2, W3)]
    rs = [(w.sum(1) / H).astype(np.float32)[:, None] for w in W[:3]]
    wa = np.asarray(w_att, np.float32)
    watt2 = np.stack([wa[:H], wa[H:]], axis=1)

    mbias = np.zeros((1, 8), np.float32)
    for i, x in enumerate(b_list):
        mbias[0, i] = float(x.mean())

    nc = _get_nc(cfg, None)
    in_maps = []
    for c in range(cfg.n_cores):
        in_maps.append(dict(
            featT=featT_all[c], embT=embT_all[c],
            w0=W[0], w1=W[1], w2=W[2], w3=W[3], watt=watt2,
            rs0=rs[0], rs1=rs[1], rs2=rs[2],
            dgidx=dg_all[c],
            ohT=oh_all[c].view(ml_dtypes.bfloat16),
            mbias=mbias,
        ))
    res = run_bass_kernel_spmd(nc, in_maps, core_ids=list(range(cfg.n_cores)),
                               trace=cfg.trace)
    n = node_ids.shape[0]
    out = np.zeros((n, D), np.float32)
    for c in range(cfg.n_cores):
        pc = perm[c]
        valid = pc >= 0
        out[pc[valid]] = res.results[c]["outN"][valid]
    return out, res


def kernel(**inputs) -> np.ndarray:
    out, _ = run(CFG, **inputs)
    return out
